# revision 46
# baseline (speedup 1.0000x reference)
"""BertAttention (T5-style relative-position bias) Trainium2 Bass kernel.

Strategy (8-way tensor parallel over heads, 2 heads/core):
  - Host pre-transposes hidden -> hT [HID, B*S] (bf16) so QKV projection
    produces qkvT [feat, tokens] directly (feat on partitions).
  - Per core: w_qkv column slice for its 2 heads, ordered
    [Q_h0|Q_h1|K_h0|K_h1|V_h0|V_h1], Q columns pre-scaled by 1/sqrt(HD).
  - Scores computed transposed: S^T[k, q]. The T5 bias is applied
    multiplicatively AFTER the exp (exp(s+b) = exp(s)*exp(b)): a single
    SBUF-resident Toeplitz band Wb[p, h, z] = exp(bias)[h, p+z] provides
    every bias tile as a view, and one DVE multiply per k-tile applies it.
    Softmax needs no max-subtraction (|s| < ~4); denominator comes from a
    ones-column appended to V.
  - Emission order pipelines the two batches: QKV(b0); attention(b0) with
    batch-1's QKV ops interleaved into the instruction stream (attention is
    Activation-bound, so the PE executes b1's projection in its idle
    slices); attention(b1); then AllToAll(b1) overlapped with dense(b0).
  - AllToAll (bf16 payload) reshards ctx^T from head-split to token-split;
    dense is computed transposed (out^T[e, t]) so b_dense is a
    per-partition bias.
  - Host reassembles out^T column chunks, transposes, reshapes.

Matmuls run bf16 (inputs pre-rounded on host) with fp32 PSUM accumulation;
Q/K activations stay float32r in SBUF. End-to-end rel err ~4e-3.
"""
import sys
import math

sys.path.insert(0, "/opt/trn_rl_repo")

import numpy as np
import ml_dtypes

import concourse.bass as bass
import concourse.bacc as bacc
import concourse.tile as tile
import concourse.mybir as mybir
from concourse.bass_utils import run_bass_kernel_spmd
from concourse.masks import make_identity

F32 = mybir.dt.float32
F32R = mybir.dt.float32r
BF16 = mybir.dt.bfloat16
Exp = mybir.ActivationFunctionType.Exp
ADD = mybir.AluOpType.add
MULT = mybir.AluOpType.mult

B, S, HID = 2, 2048, 1024
NH, HD = 16, 64
NB, MAXD = 32, 128
N_CORES = 8
HPC = NH // N_CORES          # heads per core = 2
T = B * S                    # 4096 flat tokens
FEAT = 3 * HPC * HD          # 384 qkv features per core
TC = T // 512                # 8 token chunks of 512
KTILES = S // 128            # 16 k tiles per batch
QCH = S // 512               # 4 q chunks of 512 per batch
TW = 4096                    # padded width of expanded bias table (indices 0..4094 used)
BW = 3968                    # Toeplitz band width: base in [0,3456] + 512 cols


def _bucket_map_rev():
    """rev[z] = bucket(2047 - z) for z in [0, 4094], T5 bidirectional buckets."""
    rel = (2047 - np.arange(TW - 1)).astype(np.int64)   # k - q
    nb = NB // 2                                        # 16
    base = np.where(rel > 0, nb, 0)
    r = np.abs(rel)
    max_exact = nb // 2                                 # 8
    is_small = r < max_exact
    tmp = np.log(np.maximum(r, 1).astype(np.float32) / np.float32(max_exact))
    large = tmp / np.float32(math.log(MAXD / max_exact)) * np.float32(nb - max_exact)
    large_i = max_exact + large.astype(np.int32)
    large_i = np.minimum(large_i, nb - 1)
    return (base + np.where(is_small, r, large_i)).astype(np.int32)  # [4095]


def _build_program(reps=1):
    nc = bacc.Bacc("TRN2", target_bir_lowering=False, debug=False,
                   enable_asserts=True, num_devices=N_CORES)

    hT_d = nc.dram_tensor("hT", [HID, T], BF16, kind="ExternalInput")
    wq_d = nc.dram_tensor("wq", [HID, FEAT], BF16, kind="ExternalInput")
    bq_d = nc.dram_tensor("bq", [FEAT, 1], F32, kind="ExternalInput")
    wd_d = nc.dram_tensor("wd", [HID, HID], BF16, kind="ExternalInput")
    bd_d = nc.dram_tensor("bd", [HID, 1], F32, kind="ExternalInput")
    tT_d = nc.dram_tensor("tT", [NB, HPC], F32R, kind="ExternalInput")
    oh_d = nc.dram_tensor("oh", [NB, TW], F32R, kind="ExternalInput")
    out_d = nc.dram_tensor("outT", [reps * (T // N_CORES), HID], F32,
                           kind="ExternalOutput")

    with tile.TileContext(nc) as tc:
        with tc.tile_pool(name="const", bufs=1) as cst, \
             tc.tile_pool(name="big", bufs=1) as big, \
             tc.tile_pool(name="dram", bufs=1, space="DRAM") as dram:

            # ---------------- constants (outside the rep loop) ----------------
            ident_f = cst.tile([128, 128], F32, tag="identf")
            make_identity(nc, ident_f[:])
            identr = cst.tile([128, 128], BF16, tag="identr")
            nc.vector.tensor_copy(identr[:], ident_f[:])
            ones_f = cst.tile([128, 1], F32, tag="ones")
            nc.gpsimd.memset(ones_f[:], 1.0)
            bq_sb = cst.tile([128, 3, 1], F32, tag="bq")
            nc.sync.dma_start(bq_sb[:], bq_d[:, :].rearrange("(m p) o -> p m o", p=128))

            for r in range(reps):
                # each rep writes its own output slice so that repeated
                # bodies stay live (bench timing differences rep counts)
                _emit_body(nc, tc, big, dram, identr, ones_f, bq_sb, bd_d,
                           hT_d, wq_d, wd_d, tT_d, oh_d, out_d,
                           r * (T // N_CORES))

    nc.compile()
    return nc


def _emit_qkv(nc, b, hT_d, wq_sb, bq_sb, ones_f, identr, htp, qtile, ttile,
              QT, KT, VT, Vaug, w=1024):
    """Generator emitting one batch's QKV projection + V transposes, one
    instruction per yield (so it can be interleaved into another stream).
    w is the moving-operand width (1024 halves the PE instruction count but
    needs a 2-bank PSUM slot; 512 fits in one bank)."""
    # ones columns (softmax denominator lanes) in one copy
    nc.vector.tensor_copy(
        Vaug[:].rearrange("p t (g c) -> p t g c", c=65)[:, :, :, 64:65],
        ones_f[:, 0:1].to_broadcast([128, KTILES, 2, 1]))
    yield
    dests = (QT, KT, VT)
    tpc = w // 128             # token tiles per chunk
    for tci in range(S // w):
        gci = b * (S // w) + tci
        hts = []
        for kt in range(8):
            ht = htp.tile([128, w], BF16, tag="ht", name=f"ht{b}_{tci}_{kt}")
            nc.sync.dma_start(
                ht[:], hT_d[128 * kt:128 * (kt + 1), w * gci:w * (gci + 1)])
            hts.append(ht)
            yield
        for m in range(3):
            ps = qtile(f"q{b}_{tci}_{m}")
            for kt in range(8):
                # ISA caps matmul AP dims at 512 elements: view 1024-wide
                # operands as [.., 2, 512]
                if w > 512:
                    nc.tensor.matmul(
                        ps[:].rearrange("p (a z) -> p a z", z=512),
                        wq_sb[:, kt, m * 128:(m + 1) * 128],
                        hts[kt][:].rearrange("p (a z) -> p a z", z=512),
                        start=(kt == 0), stop=(kt == 7))
                else:
                    nc.tensor.matmul(ps[:], wq_sb[:, kt, m * 128:(m + 1) * 128],
                                     hts[kt][:], start=(kt == 0), stop=(kt == 7))
                yield
            nc.vector.tensor_tensor(
                dests[m][:, w * tci:w * (tci + 1)], ps[:],
                bq_sb[:, m, 0:1].to_broadcast([128, w]), ADD)
            yield
        # V transpose for this chunk's token tiles
        for t in range(tpc * tci, tpc * (tci + 1)):
            vslot = Vaug[:, t, :].rearrange("p (g c) -> p g c", c=65)
            tp = ttile(f"tp{b}_{t}")
            nc.tensor.transpose(tp[:], VT[:, 128 * t:128 * (t + 1)], identr[:])
            yield
            nc.vector.tensor_copy(vslot[:, :, 0:64],
                                  tp[:].rearrange("p (g c) -> p g c", c=64))
            yield


def _emit_dense(nc, b, a2a_out, wd_sb, bd_bc, cfp, dtile, otile, out_d, ro,
                w=1024):
    """Generator emitting one batch's dense projection, transposed
    orientation: out[tokens, e] = cf^T @ wd, so wd (bf16, w wide) is the
    moving operand — wide matmuls instead of 128-narrow ones."""
    HB = S // N_CORES          # 256 tokens per core per batch
    cfs = []
    for j in range(8):
        cf = cfp.tile([128, HB], BF16, tag=f"cf{b}", name=f"cf{b}_{j}", bufs=8)
        nc.sync.dma_start(cf[:], a2a_out[128 * j:128 * (j + 1), :])
        cfs.append(cf)
        yield
    for th in range(HB // 128):
        for eh in range(HID // w):
            ps = dtile(f"dp{b}_{th}_{eh}")
            for j in range(8):
                if w > 512:
                    nc.tensor.matmul(
                        ps[:].rearrange("p (a z) -> p a z", z=512),
                        cfs[j][:, 128 * th:128 * (th + 1)],
                        wd_sb[:, j, w * eh:w * (eh + 1)].rearrange(
                            "p (a z) -> p a z", z=512),
                        start=(j == 0), stop=(j == 7))
                else:
                    nc.tensor.matmul(ps[:], cfs[j][:, 128 * th:128 * (th + 1)],
                                     wd_sb[:, j, w * eh:w * (eh + 1)],
                                     start=(j == 0), stop=(j == 7))
                yield
            ot = otile(f"ot{b}_{th}_{eh}")
            nc.vector.tensor_tensor(ot[:], ps[:],
                                    bd_bc[:, w * eh:w * (eh + 1)], ADD)
            yield
            nc.sync.dma_start(
                out_d[ro + b * HB + 128 * th:ro + b * HB + 128 * (th + 1),
                      w * eh:w * (eh + 1)], ot[:])
            yield


def _emit_body(nc, tc, big, dram, identr, ones_f, bq_sb, bd_d,
               hT_d, wq_d, wd_d, tT_d, oh_d, out_d, ro=0):
    # persistent tensors for this rep
    wq_sb = big.tile([128, 8, FEAT], BF16, tag="wq", name="wq_sb")
    nc.sync.dma_start(wq_sb[:], wq_d[:, :].rearrange("(j p) f -> p j f", p=128))
    QTb = [big.tile([128, S], F32R, tag=f"QT{b}", name=f"QT{b}") for b in range(B)]
    KTb = [big.tile([128, S], F32R, tag=f"KT{b}", name=f"KT{b}") for b in range(B)]
    VTb = [big.tile([128, S], BF16, tag=f"VT{b}", name=f"VT{b}") for b in range(B)]
    Vaugb = [big.tile([128, KTILES, 130], BF16, tag=f"Vaug{b}", name=f"Vaug{b}")
             for b in range(B)]
    ctxTb = [big.tile([128, S], BF16, tag=f"ctxT{b}", name=f"ctxT{b}")
             for b in range(B)]
    # Toeplitz exp(bias) band: Wb[p, h, z] = exp_trev[h, p + z]; every bias
    # tile for (qc, kt, h) is the view Wb[:, h, base:base+512] with
    # base = 1920 - 128*kt + q0 in [0, 3456]
    Wb = big.tile([128, HPC, BW], BF16, tag="Wb", name="Wb")
    wd_sb = big.tile([128, 8, HID], BF16, tag="wd", name="wd_sb")
    bd_row = big.tile([1, HID], F32, tag="bdr", name="bd_row")
    bd_bc = big.tile([128, HID], F32, tag="bdbc", name="bd_bc")
    trev = dram.tile([HPC, TW], BF16, name="trev")

    # ---------------- QKV(b0) + bias-band build ----------------
    # qkv(b0)'s first-chunk DMAs are emitted before the band build and the
    # dense-weight loads so the projection starts as soon as possible
    with tc.tile_pool(name="htp0", bufs=16) as htp0, \
         tc.tile_pool(name="qps0", bufs=4, space="PSUM") as qps0, \
         tc.tile_pool(name="tps0", bufs=2, space="PSUM") as tps0, \
         tc.tile_pool(name="txp", bufs=2, space="PSUM") as txp, \
         tc.tile_pool(name="txs", bufs=1) as txs:
        qkv0 = _emit_qkv(nc, 0, hT_d, wq_sb, bq_sb, ones_f, identr, htp0,
                         lambda n: qps0.tile([128, 512], F32, tag="qkv", name=n),
                         lambda n: tps0.tile([128, 128], BF16, tag="tr", name=n),
                         QTb[0], KTb[0], VTb[0], Vaugb[0], w=512)
        for _ in range(9):                     # ones + chunk-0 hT loads (w=512)
            next(qkv0)

        # expanded exp(bias) band (device-side gather)
        tT_sb = txs.tile([NB, HPC], F32R, tag="tT")
        nc.sync.dma_start(tT_sb[:], tT_d[:, :])
        oh_sb = txs.tile([NB, TW], F32R, tag="oh")
        nc.sync.dma_start(oh_sb[:], oh_d[:, :])
        trev_sb = txs.tile([HPC, TW], BF16, tag="trevsb")
        for i in range(TW // 512):
            tx_ps = txp.tile([HPC, 512], F32, tag="tx")
            nc.tensor.matmul(tx_ps[:], tT_sb[:], oh_sb[:, i * 512:(i + 1) * 512],
                             start=True, stop=True)
            # store exp(bias): applied multiplicatively post-softmax-exp
            nc.scalar.activation(trev_sb[:, i * 512:(i + 1) * 512], tx_ps[:], Exp)
        nc.sync.dma_start(trev[:], trev_sb[:])
        nc.sync.dma_start(
            Wb[:], bass.AP(trev.tensor, trev.offset,
                           [[1, 128], [TW, HPC], [1, BW]]))

        for _ in qkv0:
            pass

    # dense weights + replicated dense bias (needed only by the tail)
    nc.sync.dma_start(wd_sb[:], wd_d[:, :].rearrange("(j p) e -> p j e", p=128))
    nc.sync.dma_start(bd_row[:],
                      bd_d[:, :].rearrange("(p e) x -> p (e x)", p=1))
    nc.gpsimd.partition_broadcast(bd_bc[:], bd_row[:])

    # -------- attention (both batches) + hidden QKV(b1) + a2a inputs --------
    a2a_in_b = [dram.tile([HID, S // N_CORES], BF16, name=f"a2ain{b}")
                for b in range(B)]
    a2a_out_b = [dram.tile([HID, S // N_CORES], BF16, name=f"a2aout{b}")
                 for b in range(B)]
    with tc.tile_pool(name="htp1", bufs=16) as htp1, \
         tc.tile_pool(name="expp", bufs=4) as expp, \
         tc.tile_pool(name="nrm", bufs=4) as nrm, \
         tc.tile_pool(name="sps", bufs=2, space="PSUM") as sps, \
         tc.tile_pool(name="cps", bufs=3, space="PSUM") as cps, \
         tc.tile_pool(name="hps", bufs=1, space="PSUM") as hps:
        # batch-1 QKV as a generator: its ops are interleaved into batch-0's
        # Activation-bound attention stream (PE idle slices execute them);
        # dense(b0) is likewise interleaved into the back half of att(b1),
        # reusing the hps PSUM slot once the hidden QKV is done with it
        hidden = _emit_qkv(nc, 1, hT_d, wq_sb, bq_sb, ones_f, identr, htp1,
                           lambda n: hps.tile([128, 512], F32, tag="h1", name=n),
                           lambda n: hps.tile([128, 128], BF16, tag="h1", name=n),
                           QTb[1], KTb[1], VTb[1], Vaugb[1], w=512)
        dense0 = _emit_dense(nc, 0, a2a_out_b[0], wd_sb, bd_bc, htp1,
                             lambda n: hps.tile([128, 512], F32, tag="h1", name=n),
                             lambda n: htp1.tile([128, 512], F32, tag="ot",
                                                 name=n, bufs=2),
                             out_d, ro, w=512)

        for b in range(B):
            for qc in range(QCH):
                q0 = qc * 512
                ctx_ps = [cps.tile([65, 512], F32, tag="ctx", name=f"ctx{h}_{b}_{qc}")
                          for h in range(HPC)]
                for kt in range(KTILES):
                    k0 = kt * 128
                    s_ps = sps.tile([128, 1024], F32, tag="S")
                    # the two half-width QK matmuls use disjoint PE row
                    # groups 0-63 / 64-127
                    for h in range(HPC):
                        nc.tensor.matmul(s_ps[:, 512 * h:512 * (h + 1)],
                                         KTb[b][64 * h:64 * h + 64, k0:k0 + 128],
                                         QTb[b][64 * h:64 * h + 64, q0:q0 + 512],
                                         start=True, stop=True)
                    er = expp.tile([128, 1024], BF16, tag="er")
                    nc.scalar.activation(er[:], s_ps[:], Exp)
                    # T5 bias applied as exp(s)*exp(b): one DVE multiply
                    # from the SBUF band instead of PE matmuls
                    base = 1920 - k0 + q0
                    es = expp.tile([128, 1024], BF16, tag="es")
                    nc.vector.tensor_tensor(
                        es[:].rearrange("p (h z) -> p h z", z=512),
                        er[:].rearrange("p (h z) -> p h z", z=512),
                        Wb[:, :, base:base + 512], MULT)
                    for h in range(HPC):
                        nc.tensor.matmul(ctx_ps[h][:],
                                         Vaugb[b][:, kt, 65 * h:65 * h + 65],
                                         es[:, 512 * h:512 * (h + 1)],
                                         start=(kt == 0), stop=(kt == KTILES - 1))
                    if b == 0:
                        for _ in range(3):
                            next(hidden, None)
                    elif qc >= 2:
                        for _ in range(2):
                            next(dense0, None)
                for h in range(HPC):
                    recip = nrm.tile([1, 512], F32, tag="recip")
                    nc.vector.reciprocal(recip[:], ctx_ps[h][64:65, :])
                    rbb = nrm.tile([64, 512], F32, tag="rbb")
                    nc.gpsimd.partition_broadcast(rbb[:], recip[:])
                    nc.vector.tensor_tensor(
                        ctxTb[b][64 * h:64 * h + 64, q0:q0 + 512],
                        ctx_ps[h][0:64, :], rbb[:], MULT)
                # stream this chunk's a2a input shards out immediately
                nc.sync.dma_start(
                    a2a_in_b[b][:].rearrange("(j p) t -> p j t", p=128)
                    [:, 2 * qc:2 * qc + 2, :],
                    ctxTb[b][:, q0:q0 + 512].rearrange("p (j t) -> p j t", t=256))
            if b == 0:
                # drain any leftover hidden QKV(b1) ops, then batch 0's
                # all-to-all (overlaps batch-1 attention on the collective
                # cores)
                for _ in hidden:
                    pass
                nc.gpsimd.collective_compute(
                    "AllToAll", mybir.AluOpType.bypass,
                    replica_groups=[list(range(N_CORES))],
                    ins=[a2a_in_b[0][:].opt()], outs=[a2a_out_b[0][:].opt()])
        for _ in dense0:
            pass

    # ---------------- batch-1 all-to-all + dense tail ----------------
    with tc.tile_pool(name="dns", bufs=1) as dns, \
         tc.tile_pool(name="dps", bufs=4, space="PSUM") as dps:
        nc.gpsimd.collective_compute(
            "AllToAll", mybir.AluOpType.bypass,
            replica_groups=[list(range(N_CORES))],
            ins=[a2a_in_b[1][:].opt()], outs=[a2a_out_b[1][:].opt()])
        for _ in _emit_dense(nc, 1, a2a_out_b[1], wd_sb, bd_bc, dns,
                             lambda n: dps.tile([128, 512], F32, tag="d1", name=n),
                             lambda n: dns.tile([128, 512], F32, tag="ot1",
                                                name=n, bufs=2),
                             out_d, ro, w=512):
            pass


_NC_CACHE = None
_OH_CACHE = None


def _onehot_cached():
    """One-hot bucket map [NB, TW] — input-independent, built once per process."""
    global _OH_CACHE
    if _OH_CACHE is None:
        bm = _bucket_map_rev()
        oh = np.zeros((NB, TW), dtype=np.float32)
        oh[bm, np.arange(TW - 1)] = 1.0
        _OH_CACHE = oh
    return _OH_CACHE


def _get_program():
    global _NC_CACHE
    if _NC_CACHE is None:
        _NC_CACHE = _build_program()
    return _NC_CACHE


def _make_in_maps(hidden_states, w_qkv, b_qkv, w_dense, b_dense, rel_attn_table):
    hidden_states = np.asarray(hidden_states, dtype=np.float32)
    w_qkv = np.asarray(w_qkv, dtype=np.float32)
    b_qkv = np.asarray(b_qkv, dtype=np.float32)
    w_dense = np.asarray(w_dense, dtype=np.float32)
    b_dense = np.asarray(b_dense, dtype=np.float32)
    rel_attn_table = np.asarray(rel_attn_table, dtype=np.float32)

    hT = np.ascontiguousarray(hidden_states.reshape(T, HID).T).astype(
        ml_dtypes.bfloat16)                                      # [HID, T]
    oh = _onehot_cached()

    scale = np.float32(1.0 / math.sqrt(HD))
    in_maps = []
    for c in range(N_CORES):
        ha, hb = HPC * c, HPC * c + 1
        cols = []
        bias = []
        for blk, sc in ((0, scale), (1, np.float32(1.0)), (2, np.float32(1.0))):
            for h in (ha, hb):
                sl = slice(blk * HID + h * HD, blk * HID + (h + 1) * HD)
                cols.append(w_qkv[:, sl] * sc)
                bias.append(b_qkv[sl] * sc)
        wq_c = np.ascontiguousarray(np.concatenate(cols, axis=1)).astype(
            ml_dtypes.bfloat16)                                          # [HID, 384]
        bq_c = np.concatenate(bias).reshape(FEAT, 1).astype(np.float32)
        in_maps.append({
            "hT": hT,
            "wq": wq_c,
            "bq": bq_c,
            "wd": w_dense.astype(ml_dtypes.bfloat16),
            "bd": b_dense.reshape(HID, 1),
            "tT": np.ascontiguousarray(rel_attn_table[ha:hb + 1].T),     # [32, 2]
            "oh": oh,
        })
    return in_maps


def kernel(hidden_states, w_qkv, b_qkv, w_dense, b_dense, rel_attn_table):
    in_maps = _make_in_maps(hidden_states, w_qkv, b_qkv, w_dense, b_dense,
                            rel_attn_table)
    nc = _get_program()
    res = run_bass_kernel_spmd(nc, in_maps, core_ids=list(range(N_CORES)))
    HB = S // N_CORES
    full = np.empty((T, HID), dtype=np.float32)
    for c in range(N_CORES):
        o = res.results[c]["outT"]            # [2*HB, HID]: [b0 block c; b1 block c]
        full[c * HB:(c + 1) * HB] = o[:HB]
        full[S + c * HB:S + (c + 1) * HB] = o[HB:]
    return full.reshape(B, S, HID)
